# revision 20
# baseline (speedup 1.0000x reference)
"""Chamfer distance kernel for Trainium2 (8 NeuronCores, Bass/Tile).

Problem: x [4, 8192, 3], y [4, 8192, 3] f32.
  d[b,i,j] = ||x[b,i] - y[b,j]||^2
  out = mean_b mean_i min_j d  +  mean_b mean_j min_i d   (scalar f32)

Sharding: core k handles batch b = k//2, half h = k%2 of x's N dimension.
Each core computes d for its [4096 x-rows] x [all 8192 y] block via an
augmented matmul: phi = [x, 1, x^2] rows vs psi = [-2y, y^2, 1] rows, so
the PE emits exact distances.

PE strategy: inputs split hi/lo into fp16 pairs (x = xh + xl); the three
accumulation passes (hh, hl, lh; the ~2^-22 ll term is dropped) are STACKED
into a single K=15 matmul - [PHIh;PHIh;PHIl] against [PSIh;PSIl;PSIh] -
since PE cycle cost depends only on moving columns, not K. Each 512-col
matmul writes one fp32 PSUM bank with no accumulation group. Four
row-group strips (tile_position=(32c,0)) compute four j-chunks of one
i-tile concurrently into a [128, 2048] supertile.

Reduction pipeline per supertile: ScalarE extracts PSUM->SBUF (fp16),
VectorE chains the running col-min (tensor_tensor MIN into cm[128, 8192])
and the per-i row-min (tensor_scalar 4x mode with fused min-accum chained
through scalar1). Every TYPE_B-th supertile skips ScalarE: VectorE reads
the fp32 PSUM directly for both reductions, balancing the two engines. Col-min partition reduction happens on device via PE transposes +
DVE reduce. The host only combines tiny per-core outputs.
"""

import numpy as np
from contextlib import ExitStack

import concourse.bacc as bacc
import concourse.tile as tile
from concourse import mybir

B, N, M, D = 4, 8192, 8192, 3
NCORES = 8
HALF = N // 2            # x rows per core
NIT = HALF // 128        # 32 i-tiles
NBLK = M // 128          # 64 col-min blocks
JW = 2048                # j columns per supertile
NW = M // JW             # 4 windows per i-tile
NMOV = 512               # moving cols per matmul (one fp32 PSUM bank)
F32 = mybir.dt.float32
F16 = mybir.dt.float16
I16 = mybir.dt.int16
AX = mybir.AxisListType.X
MIN = mybir.AluOpType.min
ADD = mybir.AluOpType.add
SUB = mybir.AluOpType.subtract


def _build(repeat=1, loop_n=None, type_b_every=0, mode="big", dve_every=0):
    nc = bacc.Bacc("TRN2", target_bir_lowering=False, num_devices=NCORES)
    xT = nc.declare_dram_parameter("xT", [3, HALF], F32, isOutput=False)
    yT = nc.declare_dram_parameter("yT", [3, M], F32, isOutput=False)
    xq = nc.declare_dram_parameter("xq", [128, NIT * 3], F32, isOutput=False)
    yq = nc.declare_dram_parameter("yq", [128, NBLK * 3], F32, isOutput=False)
    idf = nc.declare_dram_parameter("idf", [128, 128], F32, isOutput=False)
    idh = nc.declare_dram_parameter("idh", [128, 128], F16, isOutput=False)
    rm_out = nc.declare_dram_parameter("rm_out", [128, 1], F32, isOutput=True)
    cm_out = nc.declare_dram_parameter("cm_out", [128, NBLK], F32, isOutput=True)

    with ExitStack() as ctx:
        tc = ctx.enter_context(tile.TileContext(nc))
        persist = ctx.enter_context(tc.tile_pool(name="persist", bufs=1))
        # K=15 stacked operands, replicated in rows 32c..32c+14 per strip c
        PHI3 = persist.tile([128, HALF], F16)
        PSI3 = persist.tile([128, M], F16)
        cm = persist.tile([128, M], F16)
        rm_cols = persist.tile([128, NIT], F32)
        rm_sums = persist.tile([128, 1], F32)
        cmb = persist.tile([128, NBLK], F32)
        identf = persist.tile([128, 128], F32)
        identh = persist.tile([128, 128], F16)

        nc.sync.dma_start(out=identf, in_=idf[:, :])
        nc.sync.dma_start(out=identh, in_=idh[:, :])

        # ---- prep: build f32 phi/psi, split hi/lo, stack+replicate ----
        # Row layout: PHI rows 0-2 = x, 3 = ones, 4 = x^2
        #             PSI rows 0-2 = -2*y, 3 = y^2, 4 = ones
        with tc.tile_pool(name="prep", bufs=1) as prep, \
             tc.tile_pool(name="prep_ps", bufs=1, space="PSUM") as prep_ps:
            PHIs = prep.tile([5, HALF], F32)
            PSIs = prep.tile([5, M], F32)
            ones_stage = prep.tile([1, M], F32)
            nc.vector.memset(ones_stage, 1.0)
            nc.sync.dma_start(out=PHIs[0:3, :], in_=xT[:, :])
            nc.sync.dma_start(out=PHIs[3:4, :], in_=ones_stage[0:1, 0:HALF])
            nc.sync.dma_start(out=PSIs[4:5, :], in_=ones_stage)
            yst = prep.tile([3, M], F32)
            nc.sync.dma_start(out=yst, in_=yT[:, :])
            nc.vector.tensor_scalar_mul(PSIs[0:3, :], yst, -2.0)
            xq_t = prep.tile([128, NIT * 3], F32)
            yq_t = prep.tile([128, NBLK * 3], F32)
            nc.sync.dma_start(out=xq_t, in_=xq[:, :])
            nc.sync.dma_start(out=yq_t, in_=yq[:, :])
            sqx = prep.tile([128, NIT * 3], F32)
            sqy = prep.tile([128, NBLK * 3], F32)
            nc.scalar.activation(sqx, xq_t, mybir.ActivationFunctionType.Square)
            nc.scalar.activation(sqy, yq_t, mybir.ActivationFunctionType.Square)
            x2q = prep.tile([128, NIT], F32)
            y2q = prep.tile([128, NBLK], F32)
            nc.vector.tensor_reduce(
                out=x2q, in_=sqx.rearrange("p (t d) -> p t d", d=3), axis=AX, op=ADD
            )
            nc.vector.tensor_reduce(
                out=y2q, in_=sqy.rearrange("p (t d) -> p t d", d=3), axis=AX, op=ADD
            )
            x2ps = prep_ps.tile([NIT, 128], F32)
            y2ps = prep_ps.tile([NBLK, 128], F32)
            nc.tensor.transpose(x2ps, x2q, identf)
            nc.tensor.transpose(y2ps, y2q, identf)
            x2t = prep.tile([NIT, 128], F32)
            y2t = prep.tile([NBLK, 128], F32)
            nc.scalar.copy(x2t, x2ps)
            nc.scalar.copy(y2t, y2ps)
            nc.sync.dma_start(
                out=PHIs[4:5, :].rearrange("a (t p) -> a t p", p=128), in_=x2t
            )
            nc.sync.dma_start(
                out=PSIs[3:4, :].rearrange("a (t p) -> a t p", p=128), in_=y2t
            )
            # hi/lo split (h = fp16(v); l = fp16(v - h)) staged at rows 0-9
            PHIh = prep.tile([5, HALF], F16)
            PHIl = prep.tile([5, HALF], F16)
            PSIh = prep.tile([5, M], F16)
            PSIl = prep.tile([5, M], F16)
            nc.vector.tensor_copy(PHIh, PHIs)
            nc.vector.tensor_tensor(out=PHIl, in0=PHIs, in1=PHIh, op=SUB)
            for h in range(4):
                c = slice(h * (M // 4), (h + 1) * (M // 4))
                nc.vector.tensor_copy(PSIh[:, c], PSIs[:, c])
                nc.vector.tensor_tensor(
                    out=PSIl[:, c], in0=PSIs[:, c], in1=PSIh[:, c], op=SUB
                )
            # stack into K=15 layout per strip c:
            #   PHI3 rows 32c..+4 = PHIh, +5..+9 = PHIh, +10..+14 = PHIl
            #   PSI3 rows 32c..+4 = PSIh, +5..+9 = PSIl, +10..+14 = PSIh
            for c in range(4):
                r = 32 * c
                nc.sync.dma_start(out=PHI3[r:r + 5, :], in_=PHIh)
                nc.sync.dma_start(out=PHI3[r + 5:r + 10, :], in_=PHIh)
                nc.sync.dma_start(out=PHI3[r + 10:r + 15, :], in_=PHIl)
                nc.sync.dma_start(out=PSI3[r:r + 5, :], in_=PSIh)
                nc.sync.dma_start(out=PSI3[r + 5:r + 10, :], in_=PSIl)
                nc.sync.dma_start(out=PSI3[r + 10:r + 15, :], in_=PSIh)

        # ---- main loop ----
        mm_ps = ctx.enter_context(tc.tile_pool(name="mm_ps", bufs=2, space="PSUM"))
        if mode == "big":
            ebig = ctx.enter_context(tc.tile_pool(name="ebig", bufs=2))
        else:
            ext = ctx.enter_context(tc.tile_pool(name="ext", bufs=3))
        rmv_pool = ctx.enter_context(tc.tile_pool(name="rmv", bufs=8))
        if type_b_every or mode == "noact" or dve_every:
            scratch_pool = ctx.enter_context(tc.tile_pool(name="scr", bufs=2))
        if loop_n is not None:
            ctx.enter_context(
                tc.For_i(
                    0, loop_n, 1,
                    hint_engines=(
                        mybir.EngineType.DVE,
                        mybir.EngineType.Activation,
                        mybir.EngineType.PE,
                    ),
                )
            )
        for rep in range(repeat):
            if mode == "big":
                for it in range(NIT):
                    isl = slice(it * 128, (it + 1) * 128)
                    dve_direct = dve_every and (it % dve_every == dve_every - 1)
                    if dve_direct:
                        rmw = rmv_pool.tile(
                            [128, NW], F32, tag="rmv", name=f"rmw_{rep}_{it}"
                        )
                    else:
                        eb = ebig.tile(
                            [128, M], F16, tag="eb", name=f"eb_{rep}_{it}"
                        )
                    for w in range(NW):
                        jsl = slice(w * JW, (w + 1) * JW)
                        ps = mm_ps.tile(
                            [128, JW], F32, tag="ps", name=f"ps_{rep}_{it}_{w}"
                        )
                        for c in range(JW // NMOV):
                            nc.tensor.matmul(
                                ps[:, c * NMOV:(c + 1) * NMOV],
                                PHI3[32 * c:32 * c + 15, isl],
                                PSI3[32 * c:32 * c + 15,
                                     w * JW + c * NMOV:w * JW + (c + 1) * NMOV],
                                start=True,
                                stop=True,
                                tile_position=(32 * c, 0),
                            )
                        if dve_direct:
                            # VectorE consumes PSUM directly, ScalarE skipped
                            nc.vector.tensor_tensor(
                                out=cm[:, jsl], in0=cm[:, jsl], in1=ps, op=MIN
                            )
                            scr = scratch_pool.tile(
                                [128, JW], F16, tag="scr",
                                name=f"scr_{rep}_{it}_{w}",
                            )
                            nc.vector.tensor_scalar(
                                out=scr, in0=ps, scalar1=1e30, scalar2=None,
                                op0=MIN, op1=MIN, accum_out=rmw[:, w:w + 1],
                            )
                        else:
                            nc.scalar.copy(eb[:, jsl], ps)
                    if dve_direct:
                        nc.vector.tensor_reduce(
                            out=rm_cols[:, it:it + 1], in_=rmw, axis=AX, op=MIN
                        )
                        continue
                    # one full-row col-min chain + row-min per i-tile
                    if it == 0:
                        nc.vector.tensor_copy(cm, eb)
                    else:
                        nc.vector.tensor_tensor(
                            out=cm, in0=cm, in1=eb, op=MIN
                        )
                    nc.vector.tensor_scalar(
                        out=eb, in0=eb, scalar1=1e30, scalar2=None,
                        op0=MIN, op1=MIN, accum_out=rm_cols[:, it:it + 1],
                    )
                # tails below are shared with the other modes
                nc.vector.tensor_reduce(
                    out=rm_sums, in_=rm_cols, axis=AX, op=ADD
                )
                nc.sync.dma_start(out=rm_out[:, :], in_=rm_sums)
                for bg in range(NBLK // 4):
                    pt = mm_ps.tile(
                        [128, 512], F16, tag="ps", name=f"pt_{rep}_{bg}"
                    )
                    for q in range(4):
                        blk = bg * 4 + q
                        nc.tensor.transpose(
                            pt[:, q * 128:(q + 1) * 128],
                            cm[:, blk * 128:(blk + 1) * 128],
                            identh,
                        )
                    nc.vector.tensor_reduce(
                        out=cmb[:, bg * 4:(bg + 1) * 4],
                        in_=pt.rearrange("p (q f) -> p q f", f=128),
                        axis=AX,
                        op=MIN,
                    )
                nc.sync.dma_start(out=cm_out[:, :], in_=cmb)
                continue
            sti = 0
            for it in range(NIT):
                isl = slice(it * 128, (it + 1) * 128)
                rmw = rmv_pool.tile(
                    [128, NW], F32, tag="rmv", name=f"rmw_{rep}_{it}"
                )
                for w in range(NW):
                    jsl = slice(w * JW, (w + 1) * JW)
                    ps = mm_ps.tile(
                        [128, JW], F32, tag="ps", name=f"ps_{rep}_{it}_{w}"
                    )
                    for c in range(JW // NMOV):
                        nc.tensor.matmul(
                            ps[:, c * NMOV:(c + 1) * NMOV],
                            PHI3[32 * c:32 * c + 15, isl],
                            PSI3[32 * c:32 * c + 15,
                                 w * JW + c * NMOV:w * JW + (c + 1) * NMOV],
                            start=True,
                            stop=True,
                            tile_position=(32 * c, 0),
                        )
                    sti += 1
                    if mode == "mmonly":
                        continue
                    type_b = (type_b_every and (sti % type_b_every == 0)) \
                        or mode == "noact"
                    accum = rmw[:, w:w + 1]
                    if type_b:
                        # DVE reads the fp32 PSUM directly, no ScalarE
                        if it == 0:
                            nc.vector.tensor_copy(cm[:, jsl], ps)
                        else:
                            nc.vector.tensor_tensor(
                                out=cm[:, jsl], in0=cm[:, jsl], in1=ps, op=MIN
                            )
                        scr = scratch_pool.tile(
                            [128, JW], F16, tag="scr", name=f"scr_{rep}_{it}_{w}"
                        )
                        nc.vector.tensor_scalar(
                            out=scr, in0=ps, scalar1=1e30,
                            scalar2=None, op0=MIN, op1=MIN, accum_out=accum,
                        )
                    else:
                        e = ext.tile(
                            [128, JW], F16, tag="e", name=f"e_{rep}_{it}_{w}"
                        )
                        nc.scalar.copy(e, ps)
                        if mode == "nodve":
                            continue
                        if it == 0:
                            nc.vector.tensor_copy(cm[:, jsl], e)
                        else:
                            nc.vector.tensor_tensor(
                                out=cm[:, jsl], in0=cm[:, jsl], in1=e, op=MIN
                            )
                        nc.vector.tensor_scalar(
                            out=e, in0=e, scalar1=1e30,
                            scalar2=None, op0=MIN, op1=MIN, accum_out=accum,
                        )
                if mode != "nodve":
                    # fold the NW per-window row-mins into rm_cols[:, it]
                    nc.vector.tensor_reduce(
                        out=rm_cols[:, it:it + 1], in_=rmw, axis=AX, op=MIN
                    )

            # ---- tails ----
            if mode in ("mmonly", "nodve"):
                continue
            nc.vector.tensor_reduce(out=rm_sums, in_=rm_cols, axis=AX, op=ADD)
            nc.sync.dma_start(out=rm_out[:, :], in_=rm_sums)
            for bg in range(NBLK // 4):
                pt = mm_ps.tile(
                    [128, 512], F16, tag="ps", name=f"pt_{rep}_{bg}"
                )
                for q in range(4):
                    blk = bg * 4 + q
                    nc.tensor.transpose(
                        pt[:, q * 128:(q + 1) * 128],
                        cm[:, blk * 128:(blk + 1) * 128],
                        identh,
                    )
                nc.vector.tensor_reduce(
                    out=cmb[:, bg * 4:(bg + 1) * 4],
                    in_=pt.rearrange("p (q f) -> p q f", f=128),
                    axis=AX,
                    op=MIN,
                )
            nc.sync.dma_start(out=cm_out[:, :], in_=cmb)

    nc.compile()
    return nc


_NC = None


def _get_nc():
    global _NC
    if _NC is None:
        _NC = _build()
    return _NC


_RUNNER = None


def _get_runner():
    """Build the Bass program once and return a cached jitted 8-core runner.

    Mirrors bass2jax.run_bass_via_pjrt's multi-core path, but keeps the jitted
    shard_map callable alive so repeated kernel() calls skip XLA re-tracing.
    """
    global _RUNNER
    if _RUNNER is not None:
        return _RUNNER
    import jax
    from jax.sharding import Mesh, PartitionSpec
    from jax.experimental.shard_map import shard_map
    from concourse import mybir as mb
    from concourse import bass2jax

    nc = _get_nc()
    bass2jax.install_neuronx_cc_hook()
    partition_name = (
        nc.partition_id_tensor.name if nc.partition_id_tensor else None
    )
    in_names, out_names, out_avals, zero_outs = [], [], [], []
    for alloc in nc.m.functions[0].allocations:
        if not isinstance(alloc, mb.MemoryLocationSet):
            continue
        name = alloc.memorylocations[0].name
        if alloc.kind == "ExternalInput":
            if name != partition_name:
                in_names.append(name)
        elif alloc.kind == "ExternalOutput":
            shape = tuple(alloc.tensor_shape)
            npdt = np.dtype(mb.dt.np(alloc.dtype))
            out_avals.append(jax.core.ShapedArray(shape, npdt))
            out_names.append(name)
            zero_outs.append(np.zeros(shape, npdt))

    n_params = len(in_names)
    n_outs = len(out_names)
    param_names = list(in_names)
    # donated zero-init output buffers + partition id are also bass inputs
    in_names.extend(out_names)
    if partition_name is not None:
        in_names.append(partition_name)
    donate = tuple(range(n_params, n_params + n_outs))

    def _body(*args):
        operands = list(args)
        if partition_name is not None:
            operands.append(bass2jax.partition_id_tensor())
        outs = bass2jax._bass_exec_p.bind(
            *operands,
            out_avals=tuple(out_avals),
            in_names=tuple(in_names),
            out_names=tuple(out_names),
            lowering_input_output_aliases=(),
            sim_require_finite=True,
            sim_require_nnan=True,
            nc=nc,
        )
        return tuple(outs)

    devices = jax.devices()[:NCORES]
    mesh = Mesh(np.asarray(devices), ("core",))
    in_specs = (PartitionSpec("core"),) * (n_params + n_outs)
    out_specs = (PartitionSpec("core"),) * n_outs
    fn = jax.jit(
        shard_map(
            _body, mesh=mesh, in_specs=in_specs, out_specs=out_specs,
            check_rep=False,
        ),
        donate_argnums=donate,
        keep_unused=True,
    )

    def make_zeros():
        return [
            np.zeros((NCORES * z.shape[0], *z.shape[1:]), z.dtype)
            for z in zero_outs
        ]

    def run(in_maps):
        concat_in = [
            np.concatenate([in_maps[c][n] for c in range(NCORES)], axis=0)
            for n in param_names
        ]
        out_arrs = fn(*concat_in, *make_zeros())
        return [
            {
                n: np.asarray(out_arrs[i]).reshape(
                    NCORES, *out_avals[i].shape
                )[c]
                for i, n in enumerate(out_names)
            }
            for c in range(NCORES)
        ]

    run.fn = fn
    run.mesh = mesh
    run.param_names = param_names
    run.make_zeros = make_zeros
    _RUNNER = run
    return _RUNNER


def _in_maps(x, y):
    idf = np.eye(128, dtype=np.float32)
    idh = np.eye(128, dtype=np.float16)
    maps = []
    for k in range(NCORES):
        b, h = divmod(k, 2)
        xs = x[b, h * HALF:(h + 1) * HALF]    # [4096, 3]
        ys = y[b]                              # [8192, 3]
        maps.append({
            "xT": np.ascontiguousarray(xs.T),
            "yT": np.ascontiguousarray(ys.T),
            "xq": np.ascontiguousarray(
                xs.reshape(NIT, 128, 3).transpose(1, 0, 2).reshape(128, NIT * 3)
            ),
            "yq": np.ascontiguousarray(
                ys.reshape(NBLK, 128, 3).transpose(1, 0, 2).reshape(128, NBLK * 3)
            ),
            "idf": idf,
            "idh": idh,
        })
    return maps


def _postprocess(results):
    rm_total = 0.0
    cham_y_total = 0.0
    vecs = []
    for k in range(NCORES):
        rm_total += float(results[k]["rm_out"].astype(np.float64).sum())
        vecs.append(results[k]["cm_out"].T.reshape(M))  # vec[j], j = blk*128+p
    for b in range(B):
        m = np.minimum(vecs[2 * b], vecs[2 * b + 1])
        cham_y_total += float(m.astype(np.float64).sum())
    out = rm_total / (B * N) + cham_y_total / (B * M)
    return np.float32(out)


def kernel(x, y):
    x = np.asarray(x, dtype=np.float32)
    y = np.asarray(y, dtype=np.float32)
    run = _get_runner()
    return _postprocess(run(_in_maps(x, y)))


# revision 27
# speedup vs baseline: 1.3680x; 1.3680x over previous
"""Chamfer distance kernel for Trainium2 (8 NeuronCores, Bass/Tile).

Problem: x [4, 8192, 3], y [4, 8192, 3] f32.
  d[b,i,j] = ||x[b,i] - y[b,j]||^2
  out = mean_b mean_i min_j d  +  mean_b mean_j min_i d   (scalar f32)

Sharding: core k handles batch b = k//2, half h = k%2 of x's N dimension.
Each core computes d for its [4096 x-rows] x [all 8192 y] block via an
augmented matmul: phi = [x, 1, x^2] rows vs psi = [-2y, y^2, 1] rows, so
the PE emits exact distances.

PE strategy: inputs split hi/lo into fp16 pairs (x = xh + xl); the three
accumulation passes (hh, hl, lh; the ~2^-22 ll term is dropped) are STACKED
into a single K=15 matmul - [PHIh;PHIh;PHIl] against [PSIh;PSIl;PSIh] -
since PE cycle cost depends only on moving columns, not K. Each 512-col
matmul writes one fp32 PSUM bank with no accumulation group. Four
row-group strips (tile_position=(32c,0)) compute four j-chunks of one
i-tile concurrently into a [128, 2048] supertile.

Reduction pipeline per supertile: ScalarE extracts PSUM->SBUF (fp16),
VectorE chains the running col-min (tensor_tensor MIN into cm[128, 8192])
and the per-i row-min (tensor_scalar 4x mode with fused min-accum chained
through scalar1). Every TYPE_B-th supertile skips ScalarE: VectorE reads
the fp32 PSUM directly for both reductions, balancing the two engines. Col-min partition reduction happens on device via PE transposes +
DVE reduce. The host only combines tiny per-core outputs.
"""

import numpy as np
from contextlib import ExitStack

import concourse.bacc as bacc
import concourse.tile as tile
from concourse import mybir

B, N, M, D = 4, 8192, 8192, 3
NCORES = 8
HALF = N // 2            # x rows per core
NIT = HALF // 128        # 32 i-tiles
NBLK = M // 128          # 64 col-min blocks
JW = 2048                # j columns per supertile
NW = M // JW             # 4 windows per i-tile
NMOV = 512               # moving cols per matmul (one fp32 PSUM bank)
F32 = mybir.dt.float32
F16 = mybir.dt.float16
I16 = mybir.dt.int16
AX = mybir.AxisListType.X
MIN = mybir.AluOpType.min
ADD = mybir.AluOpType.add
SUB = mybir.AluOpType.subtract


def _build(repeat=1, loop_n=None, type_b_every=0, mode="big", dve_every=0,
           gp_every=0):
    nc = bacc.Bacc("TRN2", target_bir_lowering=False, num_devices=NCORES)
    xT = nc.declare_dram_parameter("xT", [3, HALF], F32, isOutput=False)
    yT = nc.declare_dram_parameter("yT", [3, M], F32, isOutput=False)
    xq = nc.declare_dram_parameter("xq", [128, NIT * 3], F32, isOutput=False)
    yq = nc.declare_dram_parameter("yq", [128, NBLK * 3], F32, isOutput=False)
    idf = nc.declare_dram_parameter("idf", [128, 128], F32, isOutput=False)
    idh = nc.declare_dram_parameter("idh", [128, 128], F16, isOutput=False)
    rm_out = nc.declare_dram_parameter("rm_out", [128, 1], F32, isOutput=True)
    cm_out = nc.declare_dram_parameter("cm_out", [128, NBLK], F32, isOutput=True)

    with ExitStack() as ctx:
        tc = ctx.enter_context(tile.TileContext(nc))
        persist = ctx.enter_context(tc.tile_pool(name="persist", bufs=1))
        # K=15 stacked operands, replicated in rows 32c..32c+14 per strip c
        PHI3 = persist.tile([128, HALF], F16)
        PSI3 = persist.tile([128, M], F16)
        cm = persist.tile([128, M], F16)
        cm_gp = None
        if gp_every:
            cm_gp = persist.tile([128, M], F16, name="cm_gp")
        rm_cols = persist.tile([128, NIT], F32)
        rm_sums = persist.tile([128, 1], F32)
        cmb = persist.tile([128, NBLK], F32)
        identf = persist.tile([128, 128], F32)
        identh = persist.tile([128, 128], F16)

        nc.sync.dma_start(out=identf, in_=idf[:, :])
        nc.sync.dma_start(out=identh, in_=idh[:, :])

        # ---- prep: build f32 phi/psi, split hi/lo, stack+replicate ----
        # Row layout: PHI rows 0-2 = x, 3 = ones, 4 = x^2
        #             PSI rows 0-2 = -2*y, 3 = y^2, 4 = ones
        with tc.tile_pool(name="prep", bufs=1) as prep, \
             tc.tile_pool(name="prep_ps", bufs=1, space="PSUM") as prep_ps:
            PHIs = prep.tile([5, HALF], F32)
            PSIs = prep.tile([5, M], F32)
            ones_stage = prep.tile([1, M], F32)
            nc.vector.memset(ones_stage, 1.0)
            nc.sync.dma_start(out=PHIs[0:3, :], in_=xT[:, :])
            nc.sync.dma_start(out=PHIs[3:4, :], in_=ones_stage[0:1, 0:HALF])
            nc.sync.dma_start(out=PSIs[4:5, :], in_=ones_stage)
            yst = prep.tile([3, M], F32)
            nc.sync.dma_start(out=yst, in_=yT[:, :])
            nc.vector.tensor_scalar_mul(PSIs[0:3, :], yst, -2.0)
            xq_t = prep.tile([128, NIT * 3], F32)
            yq_t = prep.tile([128, NBLK * 3], F32)
            nc.sync.dma_start(out=xq_t, in_=xq[:, :])
            nc.sync.dma_start(out=yq_t, in_=yq[:, :])
            sqx = prep.tile([128, NIT * 3], F32)
            sqy = prep.tile([128, NBLK * 3], F32)
            nc.scalar.activation(sqx, xq_t, mybir.ActivationFunctionType.Square)
            nc.scalar.activation(sqy, yq_t, mybir.ActivationFunctionType.Square)
            x2q = prep.tile([128, NIT], F32)
            y2q = prep.tile([128, NBLK], F32)
            nc.vector.tensor_reduce(
                out=x2q, in_=sqx.rearrange("p (t d) -> p t d", d=3), axis=AX, op=ADD
            )
            nc.vector.tensor_reduce(
                out=y2q, in_=sqy.rearrange("p (t d) -> p t d", d=3), axis=AX, op=ADD
            )
            x2ps = prep_ps.tile([NIT, 128], F32)
            y2ps = prep_ps.tile([NBLK, 128], F32)
            nc.tensor.transpose(x2ps, x2q, identf)
            nc.tensor.transpose(y2ps, y2q, identf)
            x2t = prep.tile([NIT, 128], F32)
            y2t = prep.tile([NBLK, 128], F32)
            nc.scalar.copy(x2t, x2ps)
            nc.scalar.copy(y2t, y2ps)
            nc.sync.dma_start(
                out=PHIs[4:5, :].rearrange("a (t p) -> a t p", p=128), in_=x2t
            )
            nc.sync.dma_start(
                out=PSIs[3:4, :].rearrange("a (t p) -> a t p", p=128), in_=y2t
            )
            # hi/lo split (h = fp16(v); l = fp16(v - h)) staged at rows 0-9
            PHIh = prep.tile([5, HALF], F16)
            PHIl = prep.tile([5, HALF], F16)
            PSIh = prep.tile([5, M], F16)
            PSIl = prep.tile([5, M], F16)
            nc.vector.tensor_copy(PHIh, PHIs)
            nc.vector.tensor_tensor(out=PHIl, in0=PHIs, in1=PHIh, op=SUB)
            for h in range(4):
                c = slice(h * (M // 4), (h + 1) * (M // 4))
                nc.vector.tensor_copy(PSIh[:, c], PSIs[:, c])
                nc.vector.tensor_tensor(
                    out=PSIl[:, c], in0=PSIs[:, c], in1=PSIh[:, c], op=SUB
                )
            # stack into K=15 layout per strip c:
            #   PHI3 rows 32c..+4 = PHIh, +5..+9 = PHIh, +10..+14 = PHIl
            #   PSI3 rows 32c..+4 = PSIh, +5..+9 = PSIl, +10..+14 = PSIh
            for c in range(4):
                r = 32 * c
                nc.sync.dma_start(out=PHI3[r:r + 5, :], in_=PHIh)
                nc.sync.dma_start(out=PHI3[r + 5:r + 10, :], in_=PHIh)
                nc.sync.dma_start(out=PHI3[r + 10:r + 15, :], in_=PHIl)
                nc.sync.dma_start(out=PSI3[r:r + 5, :], in_=PSIh)
                nc.sync.dma_start(out=PSI3[r + 5:r + 10, :], in_=PSIl)
                nc.sync.dma_start(out=PSI3[r + 10:r + 15, :], in_=PSIh)

        # ---- main loop ----
        mm_ps = ctx.enter_context(tc.tile_pool(name="mm_ps", bufs=2, space="PSUM"))
        if mode == "big":
            ebig = ctx.enter_context(tc.tile_pool(name="ebig", bufs=2))
        else:
            ext = ctx.enter_context(tc.tile_pool(name="ext", bufs=3))
        rmv_pool = ctx.enter_context(tc.tile_pool(name="rmv", bufs=8))
        if type_b_every or mode == "noact" or dve_every:
            scratch_pool = ctx.enter_context(tc.tile_pool(name="scr", bufs=2))
        if loop_n is not None:
            ctx.enter_context(
                tc.For_i(
                    0, loop_n, 1,
                    hint_engines=(
                        mybir.EngineType.DVE,
                        mybir.EngineType.Activation,
                        mybir.EngineType.PE,
                    ),
                )
            )
        for rep in range(repeat):
            if mode == "big":
                if gp_every:
                    # GPSIMD runs an independent col-min chain so the DVE
                    # chain never head-of-line blocks on the slower engine
                    nc.gpsimd.memset(cm_gp, 60000.0)
                for it in range(NIT):
                    isl = slice(it * 128, (it + 1) * 128)
                    dve_direct = dve_every and (it % dve_every == dve_every - 1)
                    if dve_direct:
                        rmw = rmv_pool.tile(
                            [128, NW], F32, tag="rmv", name=f"rmw_{rep}_{it}"
                        )
                    else:
                        eb = ebig.tile(
                            [128, M], F16, tag="eb", name=f"eb_{rep}_{it}"
                        )
                    for w in range(NW):
                        jsl = slice(w * JW, (w + 1) * JW)
                        ps = mm_ps.tile(
                            [128, JW], F32, tag="ps", name=f"ps_{rep}_{it}_{w}"
                        )
                        for c in range(JW // NMOV):
                            nc.tensor.matmul(
                                ps[:, c * NMOV:(c + 1) * NMOV],
                                PHI3[32 * c:32 * c + 15, isl],
                                PSI3[32 * c:32 * c + 15,
                                     w * JW + c * NMOV:w * JW + (c + 1) * NMOV],
                                start=True,
                                stop=True,
                                tile_position=(32 * c, 0),
                            )
                        if dve_direct:
                            # VectorE consumes PSUM directly, ScalarE skipped
                            nc.vector.tensor_tensor(
                                out=cm[:, jsl], in0=cm[:, jsl], in1=ps, op=MIN
                            )
                            scr = scratch_pool.tile(
                                [128, JW], F16, tag="scr",
                                name=f"scr_{rep}_{it}_{w}",
                            )
                            nc.vector.tensor_scalar(
                                out=scr, in0=ps, scalar1=1e30, scalar2=None,
                                op0=MIN, op1=MIN, accum_out=rmw[:, w:w + 1],
                            )
                        else:
                            nc.scalar.copy(eb[:, jsl], ps)
                    if dve_direct:
                        nc.vector.tensor_reduce(
                            out=rm_cols[:, it:it + 1], in_=rmw, axis=AX, op=MIN
                        )
                        continue
                    # one full-row col-min chain + row-min per i-tile
                    if it == 0:
                        nc.vector.tensor_copy(cm, eb)
                    elif gp_every and it % gp_every == gp_every - 1:
                        nc.gpsimd.tensor_tensor(
                            out=cm_gp, in0=cm_gp, in1=eb, op=MIN
                        )
                    else:
                        nc.vector.tensor_tensor(
                            out=cm, in0=cm, in1=eb, op=MIN
                        )
                    nc.vector.tensor_scalar(
                        out=eb, in0=eb, scalar1=1e30, scalar2=None,
                        op0=MIN, op1=MIN, accum_out=rm_cols[:, it:it + 1],
                    )
                # tails below are shared with the other modes
                if gp_every:
                    nc.vector.tensor_tensor(
                        out=cm, in0=cm, in1=cm_gp, op=MIN
                    )
                nc.vector.tensor_reduce(
                    out=rm_sums, in_=rm_cols, axis=AX, op=ADD
                )
                nc.sync.dma_start(out=rm_out[:, :], in_=rm_sums)
                for bg in range(NBLK // 4):
                    pt = mm_ps.tile(
                        [128, 512], F16, tag="ps", name=f"pt_{rep}_{bg}"
                    )
                    for q in range(4):
                        blk = bg * 4 + q
                        nc.tensor.transpose(
                            pt[:, q * 128:(q + 1) * 128],
                            cm[:, blk * 128:(blk + 1) * 128],
                            identh,
                        )
                    nc.vector.tensor_reduce(
                        out=cmb[:, bg * 4:(bg + 1) * 4],
                        in_=pt.rearrange("p (q f) -> p q f", f=128),
                        axis=AX,
                        op=MIN,
                    )
                nc.sync.dma_start(out=cm_out[:, :], in_=cmb)
                continue
            sti = 0
            for it in range(NIT):
                isl = slice(it * 128, (it + 1) * 128)
                rmw = rmv_pool.tile(
                    [128, NW], F32, tag="rmv", name=f"rmw_{rep}_{it}"
                )
                for w in range(NW):
                    jsl = slice(w * JW, (w + 1) * JW)
                    ps = mm_ps.tile(
                        [128, JW], F32, tag="ps", name=f"ps_{rep}_{it}_{w}"
                    )
                    for c in range(JW // NMOV):
                        nc.tensor.matmul(
                            ps[:, c * NMOV:(c + 1) * NMOV],
                            PHI3[32 * c:32 * c + 15, isl],
                            PSI3[32 * c:32 * c + 15,
                                 w * JW + c * NMOV:w * JW + (c + 1) * NMOV],
                            start=True,
                            stop=True,
                            tile_position=(32 * c, 0),
                        )
                    sti += 1
                    if mode == "mmonly":
                        continue
                    type_b = (type_b_every and (sti % type_b_every == 0)) \
                        or mode == "noact"
                    accum = rmw[:, w:w + 1]
                    if type_b:
                        # DVE reads the fp32 PSUM directly, no ScalarE
                        if it == 0:
                            nc.vector.tensor_copy(cm[:, jsl], ps)
                        else:
                            nc.vector.tensor_tensor(
                                out=cm[:, jsl], in0=cm[:, jsl], in1=ps, op=MIN
                            )
                        scr = scratch_pool.tile(
                            [128, JW], F16, tag="scr", name=f"scr_{rep}_{it}_{w}"
                        )
                        nc.vector.tensor_scalar(
                            out=scr, in0=ps, scalar1=1e30,
                            scalar2=None, op0=MIN, op1=MIN, accum_out=accum,
                        )
                    else:
                        e = ext.tile(
                            [128, JW], F16, tag="e", name=f"e_{rep}_{it}_{w}"
                        )
                        nc.scalar.copy(e, ps)
                        if mode == "nodve":
                            continue
                        if it == 0:
                            nc.vector.tensor_copy(cm[:, jsl], e)
                        else:
                            nc.vector.tensor_tensor(
                                out=cm[:, jsl], in0=cm[:, jsl], in1=e, op=MIN
                            )
                        nc.vector.tensor_scalar(
                            out=e, in0=e, scalar1=1e30,
                            scalar2=None, op0=MIN, op1=MIN, accum_out=accum,
                        )
                if mode != "nodve":
                    # fold the NW per-window row-mins into rm_cols[:, it]
                    nc.vector.tensor_reduce(
                        out=rm_cols[:, it:it + 1], in_=rmw, axis=AX, op=MIN
                    )

            # ---- tails ----
            if mode in ("mmonly", "nodve"):
                continue
            nc.vector.tensor_reduce(out=rm_sums, in_=rm_cols, axis=AX, op=ADD)
            nc.sync.dma_start(out=rm_out[:, :], in_=rm_sums)
            for bg in range(NBLK // 4):
                pt = mm_ps.tile(
                    [128, 512], F16, tag="ps", name=f"pt_{rep}_{bg}"
                )
                for q in range(4):
                    blk = bg * 4 + q
                    nc.tensor.transpose(
                        pt[:, q * 128:(q + 1) * 128],
                        cm[:, blk * 128:(blk + 1) * 128],
                        identh,
                    )
                nc.vector.tensor_reduce(
                    out=cmb[:, bg * 4:(bg + 1) * 4],
                    in_=pt.rearrange("p (q f) -> p q f", f=128),
                    axis=AX,
                    op=MIN,
                )
            nc.sync.dma_start(out=cm_out[:, :], in_=cmb)

    nc.compile()
    return nc


_NC = None


def _get_nc():
    global _NC
    if _NC is None:
        _NC = _build()
    return _NC


_RUNNER = None


def _get_runner():
    """Build the Bass program once and return a cached jitted 8-core runner.

    Mirrors bass2jax.run_bass_via_pjrt's multi-core path, but keeps the jitted
    shard_map callable alive so repeated kernel() calls skip XLA re-tracing.
    """
    global _RUNNER
    if _RUNNER is not None:
        return _RUNNER
    import jax
    from jax.sharding import Mesh, PartitionSpec
    from jax.experimental.shard_map import shard_map
    from concourse import mybir as mb
    from concourse import bass2jax

    nc = _get_nc()
    bass2jax.install_neuronx_cc_hook()
    partition_name = (
        nc.partition_id_tensor.name if nc.partition_id_tensor else None
    )
    in_names, out_names, out_avals, zero_outs = [], [], [], []
    for alloc in nc.m.functions[0].allocations:
        if not isinstance(alloc, mb.MemoryLocationSet):
            continue
        name = alloc.memorylocations[0].name
        if alloc.kind == "ExternalInput":
            if name != partition_name:
                in_names.append(name)
        elif alloc.kind == "ExternalOutput":
            shape = tuple(alloc.tensor_shape)
            npdt = np.dtype(mb.dt.np(alloc.dtype))
            out_avals.append(jax.core.ShapedArray(shape, npdt))
            out_names.append(name)
            zero_outs.append(np.zeros(shape, npdt))

    n_params = len(in_names)
    n_outs = len(out_names)
    param_names = list(in_names)
    # donated zero-init output buffers + partition id are also bass inputs
    in_names.extend(out_names)
    if partition_name is not None:
        in_names.append(partition_name)
    donate = tuple(range(n_params, n_params + n_outs))

    def _body(*args):
        operands = list(args)
        if partition_name is not None:
            operands.append(bass2jax.partition_id_tensor())
        outs = bass2jax._bass_exec_p.bind(
            *operands,
            out_avals=tuple(out_avals),
            in_names=tuple(in_names),
            out_names=tuple(out_names),
            lowering_input_output_aliases=(),
            sim_require_finite=True,
            sim_require_nnan=True,
            nc=nc,
        )
        return tuple(outs)

    devices = jax.devices()[:NCORES]
    mesh = Mesh(np.asarray(devices), ("core",))
    in_specs = (PartitionSpec("core"),) * (n_params + n_outs)
    out_specs = (PartitionSpec("core"),) * n_outs
    fn = jax.jit(
        shard_map(
            _body, mesh=mesh, in_specs=in_specs, out_specs=out_specs,
            check_rep=False,
        ),
        donate_argnums=donate,
        keep_unused=True,
    )

    def make_zeros():
        return [
            np.zeros((NCORES * z.shape[0], *z.shape[1:]), z.dtype)
            for z in zero_outs
        ]

    def run(in_maps):
        concat_in = [
            np.concatenate([in_maps[c][n] for c in range(NCORES)], axis=0)
            for n in param_names
        ]
        out_arrs = fn(*concat_in, *make_zeros())
        return [
            {
                n: np.asarray(out_arrs[i]).reshape(
                    NCORES, *out_avals[i].shape
                )[c]
                for i, n in enumerate(out_names)
            }
            for c in range(NCORES)
        ]

    run.fn = fn
    run.mesh = mesh
    run.param_names = param_names
    run.make_zeros = make_zeros
    _RUNNER = run
    return _RUNNER


def _in_maps(x, y):
    idf = np.eye(128, dtype=np.float32)
    idh = np.eye(128, dtype=np.float16)
    maps = []
    for k in range(NCORES):
        b, h = divmod(k, 2)
        xs = x[b, h * HALF:(h + 1) * HALF]    # [4096, 3]
        ys = y[b]                              # [8192, 3]
        maps.append({
            "xT": np.ascontiguousarray(xs.T),
            "yT": np.ascontiguousarray(ys.T),
            "xq": np.ascontiguousarray(
                xs.reshape(NIT, 128, 3).transpose(1, 0, 2).reshape(128, NIT * 3)
            ),
            "yq": np.ascontiguousarray(
                ys.reshape(NBLK, 128, 3).transpose(1, 0, 2).reshape(128, NBLK * 3)
            ),
            "idf": idf,
            "idh": idh,
        })
    return maps


def _postprocess(results):
    rm_total = 0.0
    cham_y_total = 0.0
    vecs = []
    for k in range(NCORES):
        rm_total += float(results[k]["rm_out"].astype(np.float64).sum())
        vecs.append(results[k]["cm_out"].T.reshape(M))  # vec[j], j = blk*128+p
    for b in range(B):
        m = np.minimum(vecs[2 * b], vecs[2 * b + 1])
        cham_y_total += float(m.astype(np.float64).sum())
    out = rm_total / (B * N) + cham_y_total / (B * M)
    return np.float32(out)


def kernel(x, y):
    x = np.asarray(x, dtype=np.float32)
    y = np.asarray(y, dtype=np.float32)
    run = _get_runner()
    return _postprocess(run(_in_maps(x, y)))
